# revision 2
# baseline (speedup 1.0000x reference)
"""Paged-attention decode (GQA) on 8 Trainium2 NeuronCores.

Sharding: tensor-parallel over heads. Core c owns KV head c (KVH=8) and the
4 query heads in its GQA group. The KV cache is resolved, sliced per-core and
restaged by the host as bf16 (halving HBM read traffic vs f32), with the new
K/V token written in at position L-1 (so the device sees one uniform cache,
no separate new-token path). block_tables and seq_lens are folded into the
compiled graph (decode launch config). Each core runs an identical SPMD graph
with no collectives; the host concatenates the per-core output slices.

Host staging per core c (L = seq_lens[b], nt[b] = ceil(L/128) 128-token
tiles, concatenated across sequences; TOT = 128 * sum(nt)):
  - kt [128, TOT] bf16: K transposed, kt[d, off_b*128 + t] = K_b[t, d].
    Per-seq DMA slice is contiguous per partition (up to 8KB runs).
  - vp [128, TOT] bf16: V partition-major, vp[p, (off_b+o)*128 + d]
    = V_b[o*128 + p, d]. Chunk slice [:, i*128:(i+1)*128] is [t, d].
  - qh [128, B*G] bf16: queries as [d, (b, g)].

Device algorithm per core, per sequence b (tiles i = 0..nt-1):
  - DMA kt/vp strips (pure bf16, no conversion, large contiguous runs)
  - scores[t, g] per tile: matmul(lhsT=KT_tile [d,t], rhs=q [d,4]) -> PSUM
  - exp(scale*s) on ACT (PSUM -> bf16 SBUF probs); mask tail rows of the
    last tile by a per-partition mask multiply (softmax-without-max:
    scores are O(5), no overflow)
  - out^T[d, 4] += matmul(lhsT=V_tile [t,d], rhs=probs tile [t,4]), PSUM acc
  - denominator l = ones-matmul over probs, reduced per sequence on DVE
  - finalize: broadcast 1/l via a rank-1 matmul, multiply, PE-transpose to
    [(b,g), d] layout, DMA out.
"""

import numpy as np
import sys

for _p in ("/opt/trn_rl_repo",):
    if _p not in sys.path:
        sys.path.append(_p)

SCALE = 0.08838834764831845
P = 128  # partition / head-dim / token-tile size


def _build_graph(
    nt,
    rem,
    tot,
    dma_only=False,
    pipeline_pv=True,
    replay=1,
    no_dma=False,
):
    """Build the SPMD Bacc graph, specialized on per-seq tile counts.

    nt[b]  = number of 128-token tiles for seq b (>= 1, includes new token)
    rem[b] = valid tokens in the last tile (1..128)
    tot    = total columns of the staged kt/vp inputs (128 * sum(nt))
    dma_only = ablation: issue only the K/V loads (timing the memory floor)
    pipeline_pv = emit seq b's PV phase after seq b+1's score phase, so the
        exp round-trip through ScalarE doesn't stall the PE stream
    """
    import concourse.mybir as mybir
    import concourse.tile as tile
    from concourse import bacc
    from concourse.masks import make_identity

    B = len(nt)
    G = 4  # query heads per core
    MAXNT = int(max(nt))
    off = np.concatenate([[0], np.cumsum(np.asarray(nt, dtype=np.int64))])
    f32 = mybir.dt.float32
    bf16 = mybir.dt.bfloat16

    nc = bacc.Bacc(None, target_bir_lowering=False)
    kt = nc.dram_tensor("kt", [P, tot], bf16, kind="ExternalInput")  # [d, t]
    vp = nc.dram_tensor("vp", [P, tot], bf16, kind="ExternalInput")  # [p,(o d)]
    qh = nc.dram_tensor("qh", [P, B * G], bf16, kind="ExternalInput")  # [d,(b,g)]
    out = nc.dram_tensor("out", [B, G * P], f32, kind="ExternalOutput")

    with tile.TileContext(nc) as tc:
        with tc.tile_pool(name="persist", bufs=1) as persist:
            ident_f = persist.tile([P, P], f32)
            make_identity(nc, ident_f)
            ones_col_bf = persist.tile([P, 1], bf16)
            nc.vector.memset(ones_col_bf, 1.0)
            ones_row_f = persist.tile([1, P], f32)
            nc.vector.memset(ones_row_f, 1.0)
            # mask_tab[p, r] = 1.0 if p < r else 0.0 — per-partition masks for
            # the partial last tile (r = rem)
            mask_tab = persist.tile([P, P + 1], f32)
            nc.gpsimd.memset(mask_tab, 0.0)
            nc.gpsimd.affine_select(
                out=mask_tab,
                in_=mask_tab,
                compare_op=mybir.AluOpType.is_ge,
                fill=1.0,
                base=0,
                pattern=[[-1, P + 1]],
                channel_multiplier=1,
            )
            qh_bf = persist.tile([P, B * G], bf16)
            nc.gpsimd.dma_start(qh_bf[:], qh[:])
            outT = persist.tile([P, B * G], f32)  # [d, (b,g)]
            l_red = persist.tile([1, B * G], f32)
            recip = persist.tile([1, B * G], f32)
            outN = persist.tile([P, B * G], f32)
            outF = persist.tile([P, B * G], f32)

            HALF = (MAXNT + 1) // 2

            if no_dma:
                dummy_k = persist.tile([P, HALF * P], bf16)
                dummy_v = persist.tile([P, HALF * P], bf16)
                nc.vector.memset(dummy_k, 0.0)
                nc.vector.memset(dummy_v, 0.0)

            with (
                tc.tile_pool(name="kv", bufs=3) as kvpool,
                tc.tile_pool(name="sc_ps", bufs=2, space="PSUM") as scps,
                tc.tile_pool(name="probs", bufs=2) as prpool,
                tc.tile_pool(name="acc_ps", bufs=3, space="PSUM") as accps,
            ):
                state = {}

                def _load_one(src, dst_tag, b, ntb):
                    """Load ntb tiles split into two half-strips so compute
                    can start after the first strip lands."""
                    o = int(off[b])
                    strips = []
                    for s in range(0, ntb, HALF):
                        e = min(s + HALF, ntb)
                        st = kvpool.tile(
                            [P, HALF * P], bf16, tag=f"{dst_tag}{s // HALF}"
                        )
                        nc.gpsimd.dma_start(
                            st[:, : (e - s) * P],
                            src[:, (o + s) * P : (o + e) * P],
                        )
                        strips.append(st)
                    return strips

                def emit_load(b):
                    ntb = int(nt[b])
                    if no_dma:
                        return [dummy_k, dummy_k], [dummy_v, dummy_v]
                    kb = _load_one(kt, "K", b, ntb)
                    vb = _load_one(vp, "V", b, ntb)
                    return kb, vb

                def _tile_of(strips, i):
                    return strips[i // HALF][:, (i % HALF) * P : (i % HALF + 1) * P]

                def emit_scores(b, kb, vb):
                    ntb = int(nt[b])
                    scores = scps.tile([P, G * MAXNT], f32)
                    for i in range(ntb):
                        nc.tensor.matmul(
                            scores[:, G * i : G * (i + 1)],
                            lhsT=_tile_of(kb, i),
                            rhs=qh_bf[:, G * b : G * (b + 1)],
                            start=True,
                            stop=True,
                        )
                    pb = prpool.tile([P, G * MAXNT], bf16)
                    nc.scalar.activation(
                        pb[:, : G * ntb],
                        scores[:, : G * ntb],
                        mybir.ActivationFunctionType.Exp,
                        scale=SCALE,
                    )
                    r = int(rem[b])
                    if r < P:
                        nc.vector.tensor_scalar_mul(
                            pb[:, G * (ntb - 1) : G * ntb],
                            pb[:, G * (ntb - 1) : G * ntb],
                            mask_tab[:, r : r + 1],
                        )
                    state[b] = (pb, vb)

                def emit_pv(b):
                    ntb = int(nt[b])
                    pb, vb = state.pop(b)
                    lp = accps.tile([1, G * MAXNT], f32, tag="acc")
                    nc.tensor.matmul(
                        lp[:, : G * ntb],
                        lhsT=ones_col_bf,
                        rhs=pb[:, : G * ntb],
                        start=True,
                        stop=True,
                    )
                    otp = accps.tile([P, G], f32, tag="acc")
                    for i in range(ntb):
                        nc.tensor.matmul(
                            otp,
                            lhsT=_tile_of(vb, i),
                            rhs=pb[:, G * i : G * (i + 1)],
                            start=(i == 0),
                            stop=(i == ntb - 1),
                        )
                    nc.vector.tensor_copy(outT[:, G * b : G * (b + 1)], otp)
                    nc.vector.tensor_reduce(
                        l_red[0:1, G * b : G * (b + 1)],
                        lp[0:1, : G * ntb].rearrange("p (i h) -> p h i", h=G),
                        axis=mybir.AxisListType.X,
                        op=mybir.AluOpType.add,
                    )

                def emit_body():
                    if dma_only:
                        for b in range(B):
                            kb, vb = emit_load(b)
                            # tiny consumers so the loads aren't dead
                            for st_i, st in enumerate(kb + vb):
                                nc.vector.tensor_copy(
                                    outT[0:1, 4 * b + st_i : 4 * b + st_i + 1],
                                    st[0:1, 0:1],
                                )
                        nc.vector.memset(l_red, 1.0)
                    elif pipeline_pv:
                        prev = None
                        for b in range(B):
                            kb, vb = emit_load(b)
                            emit_scores(b, kb, vb)
                            if prev is not None:
                                emit_pv(prev)
                            prev = b
                        emit_pv(prev)
                    else:
                        for b in range(B):
                            kb, vb = emit_load(b)
                            emit_scores(b, kb, vb)
                            emit_pv(b)

                if replay > 1:
                    with tc.For_i(0, replay, 1):
                        emit_body()
                else:
                    emit_body()

            # ---- finalize: out = outT / l, transposed to [(b,g), d] ----
            with tc.tile_pool(name="fin_ps", bufs=1, space="PSUM") as finps:
                nc.vector.reciprocal(recip, l_red)
                bc = finps.tile([P, B * G], f32)
                nc.tensor.matmul(
                    bc, lhsT=ones_row_f, rhs=recip, start=True, stop=True
                )
                nc.vector.tensor_mul(outN, outT, bc)
                tp2 = finps.tile([P, B * G], f32)
                nc.tensor.transpose(tp2, outN, ident_f)
                nc.vector.tensor_copy(outF, tp2)
                nc.sync.dma_start(
                    out.rearrange("b (g d) -> (b g) d", g=G), outF
                )
    nc.compile()
    return nc


def _prepare(
    query, key, value, key_cache, value_cache, block_tables, seq_lens, build=True
):
    """Build the compiled SPMD graph and the per-core input shards."""
    import ml_dtypes

    bf16 = ml_dtypes.bfloat16
    query = np.asarray(query, dtype=np.float32)
    key = np.asarray(key, dtype=np.float32)
    value = np.asarray(value, dtype=np.float32)
    key_cache = np.asarray(key_cache, dtype=np.float32)
    value_cache = np.asarray(value_cache, dtype=np.float32)
    block_tables = np.asarray(block_tables)
    seq_lens = np.asarray(seq_lens)

    B, H, D = query.shape
    KVH = key.shape[1]
    NB, BS = key_cache.shape[0], key_cache.shape[1]
    S_MAX = block_tables.shape[1] * BS
    G = H // KVH
    N_CORES = 8
    assert KVH == N_CORES and D == P

    L = np.maximum(seq_lens.astype(np.int64), 1)
    nt = ((L + P - 1) // P).astype(np.int64)  # tiles incl. the new token
    rem = L - (nt - 1) * P  # valid tokens in last tile (1..128)
    off = np.concatenate([[0], np.cumsum(nt)])
    TOT = int(off[-1]) * P

    kc_flat = key_cache.reshape(NB * BS, KVH, D)
    vc_flat = value_cache.reshape(NB * BS, KVH, D)

    # Token slot ids, concatenated per sequence (nt[b]*128 tokens each; the
    # tail past L is read-but-masked padding). With arange block tables (the
    # spec's fill) slot (b, t) is just b*S_MAX + t.
    arange_ok = bool(
        np.array_equal(
            block_tables.ravel(),
            np.arange(block_tables.size, dtype=block_tables.ravel().dtype),
        )
    )
    tok_idx = np.empty(TOT, np.int64)
    for b in range(B):
        t = np.arange(int(nt[b]) * P, dtype=np.int64)
        if arange_ok:
            ids = b * S_MAX + t
        else:
            ids = block_tables[b, t // BS].astype(np.int64) * BS + t % BS
        tok_idx[off[b] * P : (off[b] + nt[b]) * P] = ids
    newpos = off[:-1] * P + (L - 1)  # new token position in the concat layout

    nc = _build_graph(nt, rem, TOT) if build else None

    in_maps = []
    for c in range(N_CORES):
        k_sel = kc_flat[tok_idx, c, :]  # [TOT, D] f32
        v_sel = vc_flat[tok_idx, c, :]
        k_sel[newpos] = key[:, c, :]
        v_sel[newpos] = value[:, c, :]
        kt_c = np.ascontiguousarray(k_sel.T.astype(bf16))
        vp_c = np.ascontiguousarray(
            v_sel.reshape(-1, P, P).transpose(1, 0, 2).reshape(P, TOT).astype(bf16)
        )
        qh_c = np.ascontiguousarray(
            query[:, c * G : (c + 1) * G, :]
            .transpose(2, 0, 1)
            .reshape(D, B * G)
            .astype(bf16)
        )
        in_maps.append({"kt": kt_c, "vp": vp_c, "qh": qh_c})
    return nc, in_maps, (B, H, D, G)


def kernel(query, key, value, key_cache, value_cache, block_tables, seq_lens):
    from concourse.bass_utils import run_bass_kernel_spmd

    nc, in_maps, (B, H, D, G) = _prepare(
        query, key, value, key_cache, value_cache, block_tables, seq_lens
    )
    res = run_bass_kernel_spmd(nc, in_maps, core_ids=list(range(len(in_maps))))
    out = np.empty((B, H * D), np.float32)
    for c in range(len(in_maps)):
        out[:, c * G * D : (c + 1) * G * D] = res.results[c]["out"]
    return out
